# revision 44
# baseline (speedup 1.0000x reference)
"""Memory-efficient multi-head attention on 8 Trainium2 NeuronCores.

Sharding: tensor-parallel over heads (4 head-groups) x data-parallel over
batch (2) = 8 cores. Core c handles head group g = c % 4 (heads 4g..4g+3,
feature slice 512) of batch b = c // 4. Each core computes its Q/K/V
projections from sliced weights, attention for its 4 heads, and a partial
out-projection y_c = ao_c @ Wo[:, gs].T; the host sums the 4 partials per
batch and adds the output bias.

All matmuls run in fp16 (same PE rate as bf16, ~3e-4 end-to-end error,
half the DMA/SBUF of fp32, and fast-weight-load eligible so LDWEIGHTS
hides behind the matmul stream). fp8 was measured and rejected: attention
output is itself an average over ~2k keys, so elementwise quantization
noise in V or exp(scores) does NOT average away (2.7% each), and the
fp8 Q/K path measures 2.3e-2 against the 2e-2 tolerance.

Softmax row-sums run OFF the PE: exp tiles accumulate on VectorE
(acc += et, 7 adds per (qb,h) block) and a single pair of ones-matmuls
per block broadcast-reduces the accumulator across partitions (2 instead
of 16 PE matmuls per block, -48us of PE time). That makes phase 2's
per-iteration PE work (~0.9us) cheaper than the exp ACTIVATE (~1.03us),
so the out-projection is interleaved INTO the attention loop as PE
filler: attention iterates qb-outer/h-inner, and once a qb block's four
heads are normalized its four seq-tiles' out-proj chunks are injected
one [128,512]-PSUM chunk per iteration, evacuated alternately on
ScalarE/VectorE (GpSimd cannot access PSUM) and DMA'd out on the sync
queue.

Each DMA ring (sync/scalar/gpsimd; vector and tensor cannot issue DMAs)
sustains ~190-256 GB/s, is 8 descriptors deep, and delivers its first
packet only ~10.8us in.  Descriptors are therefore ordered per-ring by
consumption deadline: gpsimd carries all weights (wq,wk,wv,wo), sync +
scalar alternate the first x block's chunks (one ring alone starves the
first sweep) with the small bias consts after them, and sync carries the
remaining x blocks (naturally gated by the txb double-buffer) plus the
y stores.  The ones tile is built by a local memset so ~4us of tiny
warmup matmuls can ramp the PE p-state inside the ring spin-up window
(full clock needs ~3us of continuous execution).

Rejected by measurement: gpsimd.partition_all_reduce for the softmax
sums (6.7us per call, stalls the block pipeline); gpsimd PSUM
evacuation (GPSIMD cannot access PSUM at all); delaying the first
filler pop after a fresh enqueue (neutral).  VectorE instructions cost
~685ns nearly independent of size, so plans must budget by op count.
Measured: 412us baseline -> 377us (PE 92.5% busy; remaining idle is the
~7us framework preamble, ~10.8us ring spin-up overlapped by warmup, and
~4us teardown).
"""

import sys

if "/opt/trn_rl_repo" not in sys.path:
    sys.path.insert(0, "/opt/trn_rl_repo")

from contextlib import ExitStack

import numpy as np

import concourse.bacc as bacc
import concourse.mybir as mybir
import concourse.tile as tile
from concourse.bass_utils import run_bass_kernel_spmd

B, S, D, H = 2, 2048, 2048, 16
HD = 128               # head dim
G = 4                  # head groups (tensor-parallel degree)
HPG = H // G           # heads per group = 4
FC = HPG * HD          # per-core feature slice = 512
KC = D // 128          # contraction chunks = 16
SB = 4                 # seq blocks (512 wide)
QB = 4                 # q blocks (512 wide)
ST = S // 128          # seq tiles = 16
SCALE = float(HD) ** -0.5
# fine-grained contraction-chunk sweep: each small DMA matches the matmul
# consumption rate of one sweep pass, so the projections track the
# weight/x transfers without burst stalls; a single-kc head (4 chunks)
# rides out the DMA rings' slow first descriptors
_CHUNKS = [slice(i, i + 1) for i in range(4)] + \
    [slice(2 * i, 2 * i + 2) for i in range(2, 8)]

F32 = mybir.dt.float32
F16 = mybir.dt.float16

PROFILE = False        # set by test.py to collect an NTFF trace
LAST = {}              # exec_time_ns etc. stashed here when PROFILE

_cache = {}


def _build(masked: bool):
    nc = bacc.Bacc("TRN2", target_bir_lowering=False)

    xb = nc.dram_tensor("xb", (D, S), F16, kind="ExternalInput")
    wqh = nc.dram_tensor("wqh", (D, FC), F16, kind="ExternalInput")
    wkh = nc.dram_tensor("wkh", (D, FC), F16, kind="ExternalInput")
    wvh = nc.dram_tensor("wvh", (D, FC), F16, kind="ExternalInput")
    woh = nc.dram_tensor("woh", (FC, D), F16, kind="ExternalInput")
    bq2 = nc.dram_tensor("bq2", (128, HPG), F32, kind="ExternalInput")
    bk2 = nc.dram_tensor("bk2", (128, HPG), F32, kind="ExternalInput")
    bvb = nc.dram_tensor("bvb", (128, FC), F32, kind="ExternalInput")
    mT = None
    if masked:
        mT = nc.dram_tensor("mT", (S, S), F32, kind="ExternalInput")
    y = nc.dram_tensor("y", (S, D), F16, kind="ExternalOutput")

    xb_v = xb[:].rearrange("(c p) s -> p c s", p=128)
    wq_v = wqh[:].rearrange("(c p) f -> p c f", p=128)
    wk_v = wkh[:].rearrange("(c p) f -> p c f", p=128)
    wv_v = wvh[:].rearrange("(c p) f -> p c f", p=128)
    wo_v = woh[:].rearrange("(c p) f -> p c f", p=128)
    mT_v = mT[:].rearrange("(c p) q -> p c q", p=128) if masked else None

    EXP = mybir.ActivationFunctionType.Exp
    IDN = mybir.ActivationFunctionType.Identity

    with tile.TileContext(nc) as tc, ExitStack() as top:
        const = top.enter_context(tc.tile_pool(name="const", bufs=1))
        store = top.enter_context(tc.tile_pool(name="store", bufs=1))
        attp = top.enter_context(tc.tile_pool(name="attp", bufs=1))

        t_ones = const.tile([128, 128], F16, tag="ones")
        t_bq = const.tile([128, HPG], F32, tag="bq")
        t_bk = const.tile([128, HPG], F32, tag="bk")
        t_bvb = const.tile([128, FC], F32, tag="bvb")
        # the DMA rings take ~10.8us to deliver their first packet, so the
        # ones tile is memset locally: the PE p-state warmup can then run
        # ~7..11us and the first real matmul starts at full clock
        nc.gpsimd.memset(t_ones[:], 1.0)

        QT = [store.tile([128, S], F16, tag=f"qt{h}", name=f"qt{h}")
              for h in range(HPG)]
        KT = [store.tile([128, S], F16, tag=f"kt{h}", name=f"kt{h}")
              for h in range(HPG)]
        V = [store.tile([128, FC], F16, tag=f"v{kt}", name=f"v{kt}")
             for kt in range(ST)]
        AO = [store.tile([128, S], F16, tag=f"ao{h}", name=f"ao{h}")
              for h in range(HPG)]
        WoS = [store.tile([128, D], F16, tag=f"wo{fc}", name=f"wo{fc}")
               for fc in range(HPG)]

        with tc.tile_pool(name="ps", bufs=1, space="PSUM") as psp:
            # PSUM budget (8 banks): s 2x[128,1024] (4) + av 2x[128,512]
            # (2) + smpy 2x[128,512] (2).  Phase 1's four concurrent
            # projection groups borrow [av, av, smpy, smpy].
            def groups4():
                return [psp.tile([128, 512], F32, tag=t, bufs=2,
                                 name=f"g{j}")
                        for j, t in enumerate(("av", "av", "smpy", "smpy"))]

            # PE p-state warmup: ~4us of small matmuls on the memset ones
            # tile fill the DMA-ring spin-up window; the ramp to max PE
            # clock needs ~3us of continuous execution.
            warm = psp.tile([128, 512], F32, tag="av", bufs=2, name="warm")
            for wi in range(36):
                nc.tensor.matmul(
                    warm[:, 0:128], t_ones[:], t_ones[:],
                    start=(wi == 0), stop=(wi == 35),
                )

            # attention iteration space + scores helper, defined early so
            # phase 1 can prefetch the first two score tiles (hides the
            # first exp's latency behind the last V-projection matmuls)
            iters = [(qb, h, ktp)
                     for qb in range(QB)
                     for h in range(HPG)
                     for ktp in range(8)]

            def scores(qb, h, ktp):
                t = psp.tile([128, 1024], F32, tag="s", bufs=2,
                             name="ps_s")
                qsl = slice(qb * 512, (qb + 1) * 512)
                for half in range(2):
                    kt = 2 * ktp + half
                    nc.tensor.matmul(
                        t[:, half * 512 : (half + 1) * 512],
                        KT[h][:, kt * 128 : (kt + 1) * 128],
                        QT[h][:, qsl],
                        start=True,
                        stop=True,
                    )
                return t

            pre = []

            # ---- phase 1: Q/K/V projections --------------------------
            with tc.tile_pool(name="wp", bufs=1) as wp, \
                 tc.tile_pool(name="xp", bufs=1) as xp:
                t_wq = wp.tile([128, KC, FC], F16, tag="wq")
                t_wk = wp.tile([128, KC, FC], F16, tag="wk")
                t_wv = wp.tile([128, KC, FC], F16, tag="wv")
                # chunked so partial arrival unlocks the chunk-progressive
                # matmul sweeps (a single dma_start only signals when the
                # whole transfer lands).  One ring (gpsimd) carries all
                # weights in consumption-deadline order while the other
                # (sync) carries x: each ring sustains ~190 GB/s and the
                # two together saturate HBM without stealing bandwidth
                # from earlier-deadline transfers.
                for csl in _CHUNKS:
                    nc.gpsimd.dma_start(t_wq[:, csl, :], wq_v[:, csl, :])
                for csl in _CHUNKS:
                    nc.gpsimd.dma_start(t_wk[:, csl, :], wk_v[:, csl, :])
                for csl in _CHUNKS:
                    nc.gpsimd.dma_start(t_wv[:, csl, :], wv_v[:, csl, :])
                for fc in range(HPG):
                    nc.gpsimd.dma_start(WoS[fc][:], wo_v[:, fc, :])

                for sb in range(SB):
                    ssl = slice(sb * 512, (sb + 1) * 512)
                    txb = xp.tile([128, KC, 512], F16, tag="xb", bufs=2)
                    if sb == 0:
                        # one ring can't feed the first sweep alone
                        # (~190 GB/s per ring): alternate sb0's chunks
                        # between the sync and scalar rings; the small
                        # bias consts ride scalar AFTER the x chunks
                        # (deadline order -- bvb's 256KB ahead of x cost
                        # a 5.5us PE stall)
                        for ci, csl in enumerate(_CHUNKS):
                            eng = nc.scalar if ci % 2 else nc.sync
                            eng.dma_start(txb[:, csl, :],
                                          xb_v[:, csl, ssl])
                        nc.scalar.dma_start(t_bq[:], bq2[:])
                        nc.scalar.dma_start(t_bk[:], bk2[:])
                        nc.scalar.dma_start(t_bvb[:], bvb[:])
                    else:
                        nc.sync.dma_start(txb[:], xb_v[:, :, ssl])
                    # chunk-progressive sweep over 4 concurrent PSUM groups
                    # so matmul consumption tracks the weight/x DMA arrival
                    # instead of each group demanding all 16 chunks at once
                    for wt, bias_t, dst in ((t_wq, t_bq, QT),
                                            (t_wk, t_bk, KT)):
                        pss = groups4()
                        for csl in _CHUNKS:
                            for mt in range(HPG):
                                for kc in range(csl.start, csl.stop):
                                    nc.tensor.matmul(
                                        pss[mt][:],
                                        wt[:, kc,
                                           mt * 128 : (mt + 1) * 128],
                                        txb[:, kc, :],
                                        start=(kc == 0),
                                        stop=(kc == KC - 1),
                                    )
                        for mt in range(HPG):
                            nc.scalar.activation(
                                dst[mt][:, ssl], pss[mt][:], IDN,
                                bias=bias_t[:, mt : mt + 1], scale=1.0,
                            )
                    if sb == SB - 1:
                        pre.append(scores(*iters[0]))
                        pre.append(scores(*iters[1]))
                    psv = groups4()
                    for csl in _CHUNKS:
                        for j in range(4):
                            for kc in range(csl.start, csl.stop):
                                nc.tensor.matmul(
                                    psv[j][:],
                                    txb[:, kc, j * 128 : (j + 1) * 128],
                                    t_wv[:, kc, :],
                                    start=(kc == 0),
                                    stop=(kc == KC - 1),
                                )
                    for j in range(4):
                        kt = sb * 4 + j
                        nc.vector.tensor_add(V[kt][:], psv[j][:], t_bvb[:])

            # ---- phase 2: attention + interleaved out-projection ------
            # one flat software pipeline over (qb, head, kt-pair); the
            # out-projection of a finished qb block is injected one
            # [128,512] PSUM chunk per iteration as PE filler while the
            # ACT engine works through the exp stream.
            fill = []      # pending out-proj chunks (st, dcb)
            nchunk = [0]
            pend_evac = []  # (py, st, dcb, on_scalar) awaiting copy+DMA

            def issue_chunk_mm(st, dcb, tag="smpy", on_scalar=None):
                stsl = slice(st * 128, (st + 1) * 128)
                csl = slice(dcb * 512, (dcb + 1) * 512)
                py = psp.tile([128, 512], F32, tag=tag, bufs=2,
                              name="py")
                for fc in range(HPG):
                    nc.tensor.matmul(
                        py[:],
                        AO[fc][:, stsl],
                        WoS[fc][:, csl],
                        start=(fc == 0),
                        stop=(fc == HPG - 1),
                    )
                nchunk[0] += 1
                if on_scalar is None:
                    on_scalar = bool(nchunk[0] % 2)
                pend_evac.append((py, st, dcb, on_scalar))

            def flush_evac():
                # split the PSUM evacuation between ScalarE (slack behind
                # the exp stream) and VectorE; issued after the iteration's
                # adds so the add->sm chain stays at the Vector queue head
                for py, st, dcb, on_scalar in pend_evac:
                    stsl = slice(st * 128, (st + 1) * 128)
                    csl = slice(dcb * 512, (dcb + 1) * 512)
                    yt = attp.tile([128, 512], F16, tag="yt", bufs=6)
                    if on_scalar:
                        nc.scalar.copy(yt[:], py[:])
                    else:
                        nc.vector.tensor_copy(yt[:], py[:])
                    nc.sync.dma_start(y[stsl, csl], yt[:])
                pend_evac.clear()

            def issue_chunk(st, dcb, tag="smpy", on_scalar=None):
                issue_chunk_mm(st, dcb, tag, on_scalar)
                flush_evac()

            def norm(done_hq):
                # ps_sm rows are already the broadcast key-sums
                dqb, dh, dqsl, dav, dsm = done_hq
                bcr = attp.tile([128, 512], F32, tag="bcr", bufs=2)
                nc.vector.reciprocal_approx_fast(bcr[:], dsm[:])
                nc.vector.tensor_mul(AO[dh][:, dqsl], bcr[:], dav[:])
                if dh == HPG - 1:
                    for st in range(4 * dqb, 4 * dqb + 4):
                        for dcb in range(4):
                            fill.append((st, dcb))

            cur = pre[0]
            done_hq = None
            e_prev = None
            acc = None
            ps_av = None
            for i, (qb, h, ktp) in enumerate(iters):
                qsl = slice(qb * 512, (qb + 1) * 512)
                if i == 0:
                    nxt = pre[1]
                elif i + 1 < len(iters):
                    nxt = scores(*iters[i + 1])
                else:
                    nxt = None
                if ktp == 0:
                    ps_av = psp.tile([128, 512], F32, tag="av", bufs=2,
                                     name="av")
                et = attp.tile([128, 1024], F16, tag="et", bufs=8)
                nc.scalar.activation(et[:], cur[:], EXP, scale=SCALE)
                if masked:
                    mtile = attp.tile([128, 2, 512], F32,
                                      tag="mtile", bufs=3)
                    nc.sync.dma_start(
                        mtile[:], mT_v[:, 2 * ktp : 2 * ktp + 2, qsl]
                    )
                    nc.vector.tensor_mul(
                        et[:], et[:],
                        mtile[:].rearrange("p c q -> p (c q)"),
                    )
                if done_hq is not None:
                    norm(done_hq)
                    done_hq = None
                # the filler chunk's matmuls run BEFORE attn@V so the
                # iteration's exp gains ~0.9us of extra lead time before
                # its PE consumer -- attn@V's semaphore wait is then
                # pre-satisfied and its pipeline fill stays hidden;
                # around block boundaries (ktp 0-2, 7) VectorE is busy
                # with norm/add chains, so those copies go to ScalarE
                if fill:
                    issue_chunk_mm(*fill.pop(0),
                                   on_scalar=(ktp in (0, 1, 2, 7)))
                # softmax denominator accumulates on VectorE, not the PE
                if ktp == 0:
                    e_prev = et
                elif ktp == 1:
                    acc = attp.tile([128, 1024], F16, tag="acc", bufs=2)
                    nc.vector.tensor_add(acc[:], e_prev[:], et[:])
                else:
                    nc.vector.tensor_add(acc[:], acc[:], et[:])
                for half in range(2):
                    kt = 2 * ktp + half
                    esl = slice(half * 512, (half + 1) * 512)
                    nc.tensor.matmul(
                        ps_av[:],
                        V[kt][:, h * 128 : (h + 1) * 128],
                        et[:, esl],
                        start=(kt == 0),
                        stop=(kt == ST - 1),
                    )
                if ktp == 7:
                    # broadcast-reduce the accumulator: [128,128] ones
                    # stationary, output rows are the key-sums replicated
                    # across partitions (gpsimd.partition_all_reduce was
                    # measured at 6.7us per call -- far too slow)
                    ps_sm = psp.tile([128, 512], F32, tag="smpy", bufs=2,
                                     name="sm")
                    nc.tensor.matmul(ps_sm[:], t_ones[:], acc[:, 0:512],
                                     start=True, stop=False)
                    nc.tensor.matmul(ps_sm[:], t_ones[:], acc[:, 512:1024],
                                     start=False, stop=True)
                    done_hq = (qb, h, qsl, ps_av, ps_sm)
                flush_evac()
                cur = nxt
            norm(done_hq)
            # tail drain: attention PSUM tags are free now, so rotate py
            # through av+smpy (4 bufs) to hide the evacuation WARs
            di = 0
            while fill:
                issue_chunk(*fill.pop(0),
                            tag=("av" if di % 2 else "smpy"))
                di += 1

    nc.finalize()
    return nc


def _in_maps(x, mask, Wq, bq, Wk, bk, Wv, bv, Wo, bo, masked):
    per_batch = [
        np.ascontiguousarray(x[b].T).astype(np.float16) for b in range(B)
    ]
    mTb = None
    if masked:
        mTb = [
            np.ascontiguousarray((mask[b, 0] != 0).T.astype(np.float32))
            for b in range(B)
        ]
    in_maps = []
    for c in range(8):
        g, b = c % G, c // G
        gs = slice(g * FC, (g + 1) * FC)
        m = {
            "xb": per_batch[b],
            "wqh": np.ascontiguousarray(Wq[gs].T).astype(np.float16),
            "wkh": np.ascontiguousarray(Wk[gs].T).astype(np.float16),
            "wvh": np.ascontiguousarray(Wv[gs].T).astype(np.float16),
            "woh": np.ascontiguousarray(Wo[:, gs].T).astype(np.float16),
            "bq2": np.ascontiguousarray(bq[gs].reshape(HPG, 128).T),
            "bk2": np.ascontiguousarray(bk[gs].reshape(HPG, 128).T),
            "bvb": np.tile(bv[gs][None, :], (128, 1)).astype(np.float32),
        }
        if masked:
            m["mT"] = mTb[b]
        in_maps.append(m)
    return in_maps


def kernel(x, mask, Wq, bq, Wk, bk, Wv, bv, Wo, bo):
    x = np.asarray(x, dtype=np.float32)
    mask = np.asarray(mask)
    Wq, bq = np.asarray(Wq, np.float32), np.asarray(bq, np.float32)
    Wk, bk = np.asarray(Wk, np.float32), np.asarray(bk, np.float32)
    Wv, bv = np.asarray(Wv, np.float32), np.asarray(bv, np.float32)
    Wo, bo = np.asarray(Wo, np.float32), np.asarray(bo, np.float32)

    masked = bool((mask == 0).any())
    if masked not in _cache:
        _cache[masked] = _build(masked)
    nc = _cache[masked]

    in_maps = _in_maps(x, mask, Wq, bq, Wk, bk, Wv, bv, Wo, bo, masked)

    res = run_bass_kernel_spmd(
        nc, in_maps, core_ids=list(range(8)), trace=PROFILE
    )
    if PROFILE:
        LAST["exec_time_ns"] = res.exec_time_ns
        LAST["profile_json"] = res.profile_json
        LAST["trace"] = res.instructions_and_trace

    out = np.empty((B, S, D), np.float32)
    for b in range(B):
        acc = res.results[4 * b]["y"].astype(np.float64)
        for g in range(1, G):
            acc += res.results[4 * b + g]["y"].astype(np.float64)
        out[b] = (acc + bo).astype(np.float32)
    return out
